# revision 6
# baseline (speedup 1.0000x reference)
"""NeRD pixel decoder (SIREN MLP over 5x5 local patches) on 8 trn2 cores.

Sharding: row-shard the pixel dim. Core c handles image b=c//4, rows
y0=(c%4)*32 .. y0+32 (4096 pixels). SIREN weights replicated.

Layer 0 (the 5x5 conv, 84% of FLOPs) runs in fp8-e4m3 DoubleRow matmuls at
0.5 cycles/row: per output row and 128-out-chan block, 25 taps are computed
as DR pairs (x_hi, x_lo) against stride-0-duplicated fp8 weights (x split
into hi + lo e4m3 parts on host, recovering ~11-bit input precision), plus a
weight-residual correction pass (w_lo pairs over vertically adjacent taps,
whose windows don't overlap -- overlapping DR rhs windows crash the PE) for
the first 20 taps, plus a coords pair (gx/gy baked into a slab-shaped fp8
tile, phantom zero-weight second half). Layers 1/2 and the head stay in
float32r at full PE rate. Dummy fp8 warmup matmuls run during the input DMA
lead-in so the PE p-state ramp happens on garbage data.

Everything is quantized host-side (e4m3 via ml_dtypes, f32r by mantissa
rounding); the device only multiplies exactly and accumulates in f32 PSUM.
Weight scale 2^12 and x scale 2^2 keep e4m3 operands in normal range; the
activation scale folds 2^-14 back out (sin(OMEGA*(z+b0)) via ACT bias).
"""

import numpy as np
import ml_dtypes

FC = 128      # feature channels
P = 5         # patch
HID = 256
OUT = 3
OMEGA = 30.0
B, H, W = 2, 128, 128
NCORES = 8
ROWS = H // 4            # 32 image rows per core
NPIX = ROWS * W          # 4096 pixels per core
SLABR = ROWS + 4         # 36 slab rows (2 halo each side)
SLABW = W + 4            # 132 slab cols (2 pad each side)
SLAB = SLABR * SLABW     # 4752
TP = 512                 # pixels per L1/L2/head PSUM tile (= 4 image rows)
NT = NPIX // TP          # 8 tiles per core
HB = 4                   # PSUM half-phase width for L12/head

E4 = ml_dtypes.float8_e4m3
SX = 4.0                 # x (slab/coords) pre-scale
SWT = 4096.0             # weight pre-scale
NCORR = 20               # taps getting the w_lo correction (dy 0..3, all dx)
NWARM = 56               # warmup DR matmuls during DMA lead-in

_BUILT = {}


def _build(structure="v6"):
    key = structure
    if key in _BUILT:
        return _BUILT[key]

    import concourse.tile as tile
    import concourse.mybir as mybir
    from concourse import bacc

    f32 = mybir.dt.float32
    f32r = mybir.dt.float32r
    fp8 = mybir.dt.float8e4
    Sin = mybir.ActivationFunctionType.Sin
    DR = mybir.MatmulPerfMode.DoubleRow

    nc = bacc.Bacc("TRN2", target_bir_lowering=False, debug=False)

    xs = nc.dram_tensor("xs", [128, 2 * SLAB], fp8, kind="ExternalInput").ap()
    cs = nc.dram_tensor("cs", [128, SLAB], fp8, kind="ExternalInput").ap()
    w0h = nc.dram_tensor("w0h", [128, 2 * 25 * 128], fp8,
                         kind="ExternalInput").ap()
    w0l = nc.dram_tensor("w0l", [128, 2 * NCORR * 128], fp8,
                         kind="ExternalInput").ap()
    wcp = nc.dram_tensor("wcp", [128, 2 * 2 * 128], fp8,
                         kind="ExternalInput").ap()
    w1 = nc.dram_tensor("w1", [128, 4 * 128], f32r, kind="ExternalInput").ap()
    w2 = nc.dram_tensor("w2", [128, 4 * 128], f32r, kind="ExternalInput").ap()
    w3 = nc.dram_tensor("w3", [128, 2 * OUT], f32r, kind="ExternalInput").ap()
    b0 = nc.dram_tensor("b0", [128, 2], f32, kind="ExternalInput").ap()
    b1 = nc.dram_tensor("b1", [128, 2], f32, kind="ExternalInput").ap()
    b2 = nc.dram_tensor("b2", [128, 2], f32, kind="ExternalInput").ap()
    b3 = nc.dram_tensor("b3", [OUT, 1], f32, kind="ExternalInput").ap()
    out = nc.dram_tensor("out", [OUT, NPIX], f32, kind="ExternalOutput").ap()

    with tile.TileContext(nc) as tc:
        with (
            tc.tile_pool(name="const", bufs=1) as cpool,
            tc.tile_pool(name="h", bufs=2) as hpool,
            tc.tile_pool(name="osb", bufs=1) as opool,
            tc.tile_pool(name="ps", bufs=8, space="PSUM") as pspool,
        ):
            # ---- SBUF tiles ----
            xs_t = cpool.tile([128, 2 * SLAB], fp8, tag="xs", name="xs_t")
            cs_t = cpool.tile([128, SLAB], fp8, tag="cs", name="cs_t")
            w0h_t = cpool.tile([128, 2 * 25 * 128], fp8, tag="w0h",
                               name="w0h_t")
            w0l_t = cpool.tile([128, 2 * NCORR * 128], fp8, tag="w0l",
                               name="w0l_t")
            wcp_t = cpool.tile([128, 2 * 2 * 128], fp8, tag="wcp",
                               name="wcp_t")
            w1_t = cpool.tile([128, 4 * 128], f32r, tag="w1", name="w1_t")
            w2_t = cpool.tile([128, 4 * 128], f32r, tag="w2", name="w2_t")
            w3_t = cpool.tile([128, 2 * OUT], f32r, tag="w3", name="w3_t")
            b0_t = cpool.tile([128, 2], f32, tag="b0", name="b0_t")
            b1_t = cpool.tile([128, 2], f32, tag="b1", name="b1_t")
            b2_t = cpool.tile([128, 2], f32, tag="b2", name="b2_t")
            b3_t = cpool.tile([OUT, 1], f32, tag="b3", name="b3_t")
            scr8 = cpool.tile([128, 256], fp8, tag="scr8", name="scr8")
            scro = cpool.tile([128, 128], f32, tag="scro", name="scro")
            out_sb = opool.tile([OUT, NPIX], f32, tag="osb")

            # ---- input DMAs, priority order; in-order queues + range-based
            # deps let compute start as soon as its rows/blocks land ----
            def slab_rows(ap3, r0, r1):
                return ap3[:, :, r0 * SLABW:r1 * SLABW]

            xs3 = xs.rearrange("p (s n) -> p s n", s=2)
            xst3 = xs_t[:].rearrange("p (s n) -> p s n", s=2)
            nc.sync.dma_start(w0h_t[:, 0:3200], w0h[:, 0:3200])
            nc.sync.dma_start(w0l_t[:, 0:NCORR * 128], w0l[:, 0:NCORR * 128])
            nc.sync.dma_start(wcp_t[:], wcp[:])
            nc.sync.dma_start(b0_t[:], b0[:])
            row_chunks = [(0, 8), (8, 16), (16, 24), (24, 32), (32, 36)]
            for (r0, r1) in row_chunks:
                nc.sync.dma_start(slab_rows(xst3, r0, r1),
                                  slab_rows(xs3, r0, r1))
                nc.sync.dma_start(cs_t[:, r0 * SLABW:r1 * SLABW],
                                  cs[:, r0 * SLABW:r1 * SLABW])
            nc.sync.dma_start(w0h_t[:, 3200:6400], w0h[:, 3200:6400])
            nc.sync.dma_start(w0l_t[:, NCORR * 128:], w0l[:, NCORR * 128:])
            nc.sync.dma_start(w1_t[:], w1[:])
            nc.sync.dma_start(w2_t[:], w2[:])
            nc.sync.dma_start(w3_t[:], w3[:])
            nc.sync.dma_start(b1_t[:], b1[:])
            nc.sync.dma_start(b2_t[:], b2[:])
            nc.sync.dma_start(b3_t[:], b3[:])

            # ---- PE warmup on scratch data (p-state ramp during DMA) ----
            nc.vector.memset(scr8[:], 0.0)
            scr3 = scr8[:].rearrange("p (s n) -> p s n", s=2)
            for i in range(NWARM):
                psw = pspool.tile([128, 128], f32, tag="ps", name=f"psw{i}")
                nc.tensor.matmul(psw[:], scr3, scr3, start=True, stop=True,
                                 perf_mode=DR)
                if i == NWARM - 1:
                    nc.scalar.activation(scro[:], psw[:], Sin,
                                         bias=b0_t[:, 0:1], scale=1.0)

            # ---- layer 0: fp8 DR, one [128,128] psum per (m, output row) ---
            h0 = hpool.tile([128, 2 * NPIX], f32r, tag="h")
            xr = xst3  # [p, 2(hi/lo), slab]

            def tap_window(y, dy, dx):
                # [p, 2(hi/lo), 128] windows at tap (dy,dx) for output row y
                a = xr[:, :, (y + dy) * SLABW + dx:(y + dy) * SLABW + dx + 128]
                return a

            def w0h_blk(m, k):
                # stride-0 pair of the (m, tap k) hi-weight block
                a = w0h_t[:, m * 3200 + k * 128:m * 3200 + (k + 1) * 128]
                a = a.unsqueeze(1).copy()
                a.ap[1] = [0, 2]
                return a

            act_scale = OMEGA / (SX * SWT)
            for m in range(2):
                for y in range(ROWS):
                    ps = pspool.tile([128, 128], f32, tag="ps",
                                     name=f"ps_l0_{m}_{y}")
                    # coords pair: window rows y+2,y+3 of cs; half 1 weights 0
                    ca = cs_t[:, (y + 2) * SLABW + 2:(y + 2) * SLABW + 130]
                    ca = ca.unsqueeze(1).copy()
                    ca.ap[1] = [SLABW, 2]
                    nc.tensor.matmul(
                        ps[:], wcp_t[:].rearrange("p (m t c) -> p (m t) c",
                                                  m=2, t=2)[:, 2 * m:2 * m + 2, :],
                        ca, start=True, stop=False, perf_mode=DR)
                    # 25 taps: (x_hi, x_lo) against stride-0 wh
                    for k in range(25):
                        dy, dx = divmod(k, 5)
                        nc.tensor.matmul(
                            ps[:], w0h_blk(m, k), tap_window(y, dy, dx),
                            start=False, stop=False, perf_mode=DR)
                    # w_lo correction: vertical tap pairs (dy,2q)+(dy..)
                    for dx in range(5):
                        for q in range(NCORR // 10):
                            k0 = (2 * q) * 5 + dx      # tap (2q, dx)
                            blk = NCORR * 128 * m + (dx * (NCORR // 10) + q) * 256
                            lhs = w0l_t[:, blk:blk + 256].rearrange(
                                "p (t c) -> p t c", t=2)
                            rhs = xr[:, 0:1,
                                     (y + 2 * q) * SLABW + dx:
                                     (y + 2 * q) * SLABW + dx + 128].copy()
                            rhs.ap[1] = [SLABW, 2]
                            last = (dx == 4 and q == NCORR // 10 - 1)
                            nc.tensor.matmul(ps[:], lhs, rhs, start=False,
                                             stop=last, perf_mode=DR)
                    nc.scalar.activation(
                        h0[:, m * NPIX + y * 128:m * NPIX + (y + 1) * 128],
                        ps[:], Sin, bias=b0_t[:, m:m + 1], scale=act_scale)

            # ---- layers 1, 2: dense 256->256 in f32r, sin ----
            def h_slice(h, k, t):
                return h[:, k * NPIX + t * TP:k * NPIX + (t + 1) * TP]

            hin = h0
            for (wl_t, bl_t) in ((w1_t, b1_t), (w2_t, b2_t)):
                hout = hpool.tile([128, 2 * NPIX], f32r, tag="h")
                for m in range(2):
                    for th in range(0, NT, HB):
                        pss = []
                        for _i in range(HB):
                            ps_i = pspool.tile([128, TP], f32, tag="ps")
                            pss.append(ps_i)
                        for k in range(2):
                            for ti in range(HB):
                                nc.tensor.matmul(
                                    pss[ti][:],
                                    wl_t[:, (k * 2 + m) * 128:
                                         (k * 2 + m + 1) * 128],
                                    h_slice(hin, k, th + ti),
                                    start=(k == 0), stop=(k == 1))
                        for ti in range(HB):
                            nc.scalar.activation(
                                h_slice(hout, m, th + ti), pss[ti][:], Sin,
                                bias=bl_t[:, m:m + 1], scale=OMEGA)
                hin = hout

            # ---- head: 256 -> 3, + bias; chunked output DMA ----
            for th in range(0, NT, HB):
                pss = []
                for _i in range(HB):
                    ps_i = pspool.tile([OUT, TP], f32, tag="ps")
                    pss.append(ps_i)
                for k in range(2):
                    for ti in range(HB):
                        nc.tensor.matmul(
                            pss[ti][:], w3_t[:, k * OUT:(k + 1) * OUT],
                            h_slice(hin, k, th + ti), start=(k == 0),
                            stop=(k == 1))
                for ti in range(HB):
                    t = th + ti
                    nc.vector.tensor_scalar_add(
                        out_sb[:, t * TP:(t + 1) * TP], pss[ti][:],
                        b3_t[:, 0:1])
                nc.sync.dma_start(
                    out[:, th * TP:(th + HB) * TP],
                    out_sb[:, th * TP:(th + HB) * TP])

    nc.finalize()
    _BUILT[key] = nc
    return nc


def _to_f32r(a):
    """Round fp32 to the fp32r format the PE expects (low 12 mantissa bits 0)."""
    b = np.ascontiguousarray(a, np.float32).view(np.uint32).astype(np.uint64)
    r = ((b + 0x800) & 0xFFFFF000).astype(np.uint32)
    return r.view(np.float32).reshape(np.asarray(a).shape)


def _e4(a):
    return np.ascontiguousarray(a, np.float32).astype(E4)


def _prep_core_inputs(c, xi, gx, gy):
    b = c // 4
    y0 = (c % 4) * ROWS
    slab = np.zeros((128, SLABR, SLABW), np.float32)
    ylo, yhi = y0 - 2, y0 + ROWS + 2
    slo, shi = max(ylo, 0), min(yhi, H)
    slab[:, slo - ylo: shi - ylo, 2:2 + W] = xi[b, :, slo:shi, :]
    slab *= SX
    xh = _e4(slab)
    xl = _e4(slab - xh.astype(np.float32))

    csl = np.zeros((128, SLABR, SLABW), np.float32)
    csl[0, 2:35, 2:130] = SX * gx[None, :]
    # gy per slab row r (used at window row y'+2 -> image row y0+y'):
    for r in range(2, 35):
        csl[1, r, 2:130] = SX * gy[min(max(y0 + r - 2, 0), H - 1)]

    return {
        "xs": np.concatenate(
            [xh.reshape(128, SLAB), xl.reshape(128, SLAB)], axis=1),
        "cs": _e4(csl.reshape(128, SLAB)),
    }


def kernel(**inputs):
    from concourse.bass_utils import run_bass_kernel_spmd

    xi = np.asarray(inputs["xi"], np.float32)
    W0 = np.asarray(inputs["W0"], np.float32)
    b0 = np.asarray(inputs["b0"], np.float32)
    W1 = np.asarray(inputs["W1"], np.float32)
    b1 = np.asarray(inputs["b1"], np.float32)
    W2 = np.asarray(inputs["W2"], np.float32)
    b2 = np.asarray(inputs["b2"], np.float32)
    W3 = np.asarray(inputs["W3"], np.float32)
    b3 = np.asarray(inputs["b3"], np.float32)

    # ---- weight prep (replicated) ----
    # patch rows of W0 are (c, dy, dx)-ordered; ktile k=(dy*5+dx) gathers
    # rows c*25+k. Scale 2^12, split hi/lo in e4m3.
    Wp = (SWT * W0[:FC * P * P]).reshape(128, 25, HID)   # [c, k, out]
    wh_f = _e4(Wp).astype(np.float32)
    wl_f = Wp - wh_f
    # w0h layout: [m][k][128] along free dim
    w0h = np.empty((128, 2, 25, 128), np.float32)
    for m in range(2):
        for k in range(25):
            w0h[:, m, k, :] = wh_f[:, k, m * 128:(m + 1) * 128]
    # w0l layout: [m][dx*(NCORR//10)+q][2 taps][128]
    w0l_list = np.empty((128, 2, 5, NCORR // 10, 2, 128), np.float32)
    for m in range(2):
        for dx in range(5):
            for q in range(NCORR // 10):
                for j in range(2):
                    k = (2 * q + j) * 5 + dx
                    w0l_list[:, m, dx, q, j, :] = \
                        wl_f[:, k, m * 128:(m + 1) * 128]
    # coords weight pad: [m][2 halves][128]; half0 rows 0,1 = SWT*Wc
    Wc = SWT * W0[FC * P * P:]                            # [2, 256]
    wcp = np.zeros((128, 2, 2, 128), np.float32)
    for m in range(2):
        wcp[0:2, m, 0, :] = Wc[:, m * 128:(m + 1) * 128]

    w1_h = _to_f32r(
        W1.reshape(2, 128, 2, 128).transpose(1, 0, 2, 3).reshape(128, 512))
    w2_h = _to_f32r(
        W2.reshape(2, 128, 2, 128).transpose(1, 0, 2, 3).reshape(128, 512))
    w3_h = _to_f32r(
        W3.reshape(2, 128, OUT).transpose(1, 0, 2).reshape(128, 2 * OUT))
    b0_h = np.ascontiguousarray((OMEGA * b0).reshape(2, 128).T)
    b1_h = np.ascontiguousarray((OMEGA * b1).reshape(2, 128).T)
    b2_h = np.ascontiguousarray((OMEGA * b2).reshape(2, 128).T)
    b3_h = np.ascontiguousarray(b3.reshape(OUT, 1))

    ys = np.linspace(-1.0, 1.0, H, dtype=np.float32)
    xcs = np.linspace(-1.0, 1.0, W, dtype=np.float32)

    shared = {
        "w0h": _e4(w0h.reshape(128, 2 * 25 * 128)),
        "w0l": _e4(w0l_list.reshape(128, 2 * 5 * (NCORR // 10) * 2 * 128)),
        "wcp": _e4(wcp.reshape(128, 512)),
        "w1": w1_h, "w2": w2_h, "w3": w3_h,
        "b0": b0_h, "b1": b1_h, "b2": b2_h, "b3": b3_h,
    }
    in_maps = []
    for c in range(NCORES):
        m = _prep_core_inputs(c, xi, xcs, ys)
        m.update(shared)
        in_maps.append(m)

    nc = _build()
    res = run_bass_kernel_spmd(nc, in_maps, core_ids=list(range(NCORES)))
    global LAST_RES
    LAST_RES = res

    full = np.empty((B, OUT, H, W), np.float32)
    for c in range(NCORES):
        b = c // 4
        y0 = (c % 4) * ROWS
        full[b, :, y0:y0 + ROWS, :] = res.results[c]["out"].reshape(
            OUT, ROWS, W)
    return full


# revision 12
# speedup vs baseline: 1.0715x; 1.0715x over previous
"""NeRD pixel decoder (SIREN MLP over 5x5 local patches) on 8 trn2 cores.

Sharding: row-shard the pixel dim. Core c handles image b=c//4, rows
y0=(c%4)*32 .. y0+32 (4096 pixels). SIREN weights replicated.

Layer 0 (the 5x5 conv, 84% of FLOPs) runs in fp8-e4m3 DoubleRow matmuls at
0.5 cycles/row: per output row and 128-out-chan block, 25 taps are computed
as DR pairs (x_hi, x_lo) against stride-0-duplicated fp8 weights (x split
into hi + lo e4m3 parts on host, recovering ~11-bit input precision), plus a
weight-residual correction pass (w_lo pairs over vertically adjacent taps,
whose windows don't overlap -- overlapping DR rhs windows crash the PE) for
the first NCORR taps, plus a coords pair (gx/gy baked into a slab-shaped fp8
plane, phantom zero-weight second half). Layers 1/2 and the head stay in
float32r at full PE rate.

v8 pipeline: ten input DMAs total (hi/lo/coords slabs ride one tensor in
row chunks; wcp+b0 ride the w0 tensors via byte-packing; w1/w2/w3/b1/b2/b3
ride one f32r "wtail" tensor with bitcast bias slices), split across both
HWDGE queues in deadline order -- each dma_start costs ~650ns of sequencer
+ HWDGE issue, which dominates the lead-in, so fewer is faster. Dummy fp8
warmup matmuls cover the DMA lead-in and the PE p-state ramp. m=1 row units
lag m=0 by two rows so the m=1 weight DMA can land later. L1/L2/head tiles
are fused into the L0 row stream with a one-tile stagger (and a split final
tile) so cross-engine dependencies are old when the PE reaches them and the
drain tail is short. Per-tile output DMA.

Everything is quantized host-side (e4m3 via ml_dtypes, f32r by mantissa
rounding); the device only multiplies exactly and accumulates in f32 PSUM.
Weight scale 2^12 and x scale 2^2 keep e4m3 operands in normal range; the
activation scale folds 2^-14 back out (sin(OMEGA*(z+b0)) via ACT bias).
"""

import numpy as np
import ml_dtypes

FC = 128      # feature channels
P = 5         # patch
HID = 256
OUT = 3
OMEGA = 30.0
B, H, W = 2, 128, 128
NCORES = 8
ROWS = H // 4            # 32 image rows per core
NPIX = ROWS * W          # 4096 pixels per core
SLABR = ROWS + 4         # 36 slab rows (2 halo each side)
SLABW = W + 4            # 132 slab cols (2 pad each side)
SLAB = SLABR * SLABW     # 4752
TP = 512                 # pixels per L1/L2/head PSUM tile (= 4 image rows)
NT = NPIX // TP          # 8 tiles per core

E4 = ml_dtypes.float8_e4m3
SX = 4.0                 # x (slab/coords) pre-scale
SWT = 4096.0             # weight pre-scale
NCORR = 16               # taps getting the w_lo correction (8 vertical pairs)
NWARM = 30               # warmup DR matmuls during DMA lead-in
WPAIRS = [(dx, q) for q in range(2) for dx in range(5)][:NCORR // 2]
NC128 = NCORR * 128
WTAIL = 1036             # packed w1|w2|w3|b1|b2|b3 columns (f32 each)

_BUILT = {}


def _build(structure="v8"):
    key = structure
    if key in _BUILT:
        return _BUILT[key]

    import concourse.tile as tile
    import concourse.mybir as mybir
    from concourse import bacc

    f32 = mybir.dt.float32
    f32r = mybir.dt.float32r
    fp8 = mybir.dt.float8e4
    Sin = mybir.ActivationFunctionType.Sin
    DR = mybir.MatmulPerfMode.DoubleRow

    nc = bacc.Bacc("TRN2", target_bir_lowering=False, debug=False)

    xs = nc.dram_tensor("xs", [128, 3 * SLAB], fp8, kind="ExternalInput").ap()
    w0h = nc.dram_tensor("w0h", [128, 512 + 6400], fp8,
                         kind="ExternalInput").ap()
    w0l = nc.dram_tensor("w0l", [128, 8 + 2 * NC128], fp8,
                         kind="ExternalInput").ap()
    wt = nc.dram_tensor("wt", [128, WTAIL], f32r, kind="ExternalInput").ap()
    out = nc.dram_tensor("out", [OUT, NPIX], f32, kind="ExternalOutput").ap()

    with tile.TileContext(nc) as tc:
        with (
            tc.tile_pool(name="const", bufs=1) as cpool,
            tc.tile_pool(name="h", bufs=3) as hpool,
            tc.tile_pool(name="osb", bufs=1) as opool,
            tc.tile_pool(name="ps", bufs=8, space="PSUM") as pspool,
        ):
            # ---- SBUF tiles ----
            xs_t = cpool.tile([128, 3 * SLAB], fp8, tag="xs", name="xs_t")
            w0h_t = cpool.tile([128, 512 + 6400], fp8, tag="w0h",
                               name="w0h_t")
            w0l_t = cpool.tile([128, 8 + 2 * NC128], fp8, tag="w0l",
                               name="w0l_t")
            wt_t = cpool.tile([128, WTAIL], f32r, tag="wt", name="wt_t")
            scr8 = cpool.tile([128, 256], fp8, tag="scr8", name="scr8")
            scro = cpool.tile([128, 128], f32, tag="scro", name="scro")
            out_sb = opool.tile([OUT, NPIX], f32, tag="osb")

            # packed views
            b0_v = w0l_t[:, 0:8].bitcast(f32)          # [128, 2]
            w1_v = wt_t[:, 0:512]
            w2_v = wt_t[:, 512:1024]
            w3_v = wt_t[:, 1024:1030]
            b1_v = wt_t[:, 1030:1032].bitcast(f32)
            b2_v = wt_t[:, 1032:1034].bitcast(f32)
            b3_v = wt_t[:][0:OUT, 1034:1035].bitcast(f32)   # [3, 1]

            xs3 = xs.rearrange("p (s n) -> p s n", s=3)
            xst3 = xs_t[:].rearrange("p (s n) -> p s n", s=3)

            def slab_rows(ap3, r0, r1):
                return ap3[:, :, r0 * SLABW:r1 * SLABW]

            # ---- input DMAs: deadline order, two HWDGE queues ----
            nc.sync.dma_start(w0h_t[:, 0:3712], w0h[:, 0:3712])  # wcp + m0
            nc.sync.dma_start(w0l_t[:, 0:8 + NC128],
                              w0l[:, 0:8 + NC128])               # b0 + m0
            nc.sync.dma_start(w0h_t[:, 3712:6912], w0h[:, 3712:6912])  # m1
            nc.sync.dma_start(w0l_t[:, 8 + NC128:], w0l[:, 8 + NC128:])
            nc.sync.dma_start(slab_rows(xst3, 8, 16), slab_rows(xs3, 8, 16))
            nc.sync.dma_start(slab_rows(xst3, 16, 24), slab_rows(xs3, 16, 24))
            nc.scalar.dma_start(slab_rows(xst3, 0, 8), slab_rows(xs3, 0, 8))
            nc.scalar.dma_start(slab_rows(xst3, 24, 32), slab_rows(xs3, 24, 32))
            nc.scalar.dma_start(slab_rows(xst3, 32, 36), slab_rows(xs3, 32, 36))
            nc.scalar.dma_start(wt_t[:], wt[:])

            # ---- PE warmup on scratch data (p-state ramp during DMA) ----
            nc.vector.memset(scr8[:], 0.0)
            scr3 = scr8[:].rearrange("p (s n) -> p s n", s=2)
            for i in range(NWARM):
                psw = pspool.tile([128, 128], f32, tag="ps", name=f"psw{i}")
                nc.tensor.matmul(psw[:], scr3, scr3, start=True, stop=True,
                                 perf_mode=DR)
                if i == NWARM - 1:
                    nc.scalar.activation(scro[:], psw[:], Sin,
                                         bias=b0_v[:, 0:1], scale=1.0)

            # ---- fused pipeline ----
            h0 = hpool.tile([128, 2 * NPIX], f32r, tag="h", name="h0")
            h1 = hpool.tile([128, 2 * NPIX], f32r, tag="h", name="h1")
            h2 = hpool.tile([128, 2 * NPIX], f32r, tag="h", name="h2")
            act_scale = OMEGA / (SX * SWT)

            def w0h_blk(m, k):
                off = 512 + m * 3200 + k * 128
                a = w0h_t[:, off:off + 128].unsqueeze(1).copy()
                a.ap[1] = [0, 2]   # stride-0: same hi-weights for both halves
                return a

            def emit_l0_unit(m, y):
                ps = pspool.tile([128, 128], f32, tag="ps",
                                 name=f"ps_l0_{m}_{y}")
                for k in range(25):
                    dy, dx = divmod(k, 5)
                    off = (y + dy) * SLABW + dx
                    nc.tensor.matmul(ps[:], w0h_blk(m, k),
                                     xst3[:, 0:2, off:off + 128],
                                     start=(k == 0), stop=False, perf_mode=DR)
                for pi, (dx, q) in enumerate(WPAIRS):
                    blk = 8 + NC128 * m + pi * 256
                    lhs = w0l_t[:, blk:blk + 256].rearrange(
                        "p (t c) -> p t c", t=2)
                    off = (y + 2 * q) * SLABW + dx
                    rhs = xst3[:, 0:1, off:off + 128].copy()
                    rhs.ap[1] = [SLABW, 2]       # taps (2q,dx), (2q+1,dx)
                    nc.tensor.matmul(ps[:], lhs, rhs, start=False,
                                     stop=False, perf_mode=DR)
                # coords pair last: cs plane rows y+2,y+3; half-1 weights 0
                coff = 2 * SLAB + (y + 2) * SLABW + 2
                ca = xs_t[:, coff:coff + 128].unsqueeze(1).copy()
                ca.ap[1] = [SLABW, 2]
                nc.tensor.matmul(
                    ps[:], w0h_t[:].rearrange(
                        "p (g c) -> p g c", c=128)[:, 2 * m:2 * m + 2, :],
                    ca, start=False, stop=True, perf_mode=DR)
                nc.scalar.activation(
                    h0[:, m * NPIX + y * 128:m * NPIX + (y + 1) * 128],
                    ps[:], Sin, bias=b0_v[:, m:m + 1], scale=act_scale)

            def emit_dense(lname, hin, hout, wl_v, bl_v, px0, npx):
                for m in range(2):
                    ps = pspool.tile([128, npx], f32, tag="ps",
                                     name=f"ps_{lname}_{m}_{px0}")
                    for k in range(2):
                        nc.tensor.matmul(
                            ps[:],
                            wl_v[:, (k * 2 + m) * 128:(k * 2 + m + 1) * 128],
                            hin[:, k * NPIX + px0:k * NPIX + px0 + npx],
                            start=(k == 0), stop=(k == 1))
                    nc.scalar.activation(
                        hout[:, m * NPIX + px0:m * NPIX + px0 + npx], ps[:],
                        Sin, bias=bl_v[:, m:m + 1], scale=OMEGA)

            def emit_head(px0, npx):
                ps = pspool.tile([OUT, npx], f32, tag="ps",
                                 name=f"ps_hd_{px0}")
                for k in range(2):
                    nc.tensor.matmul(
                        ps[:], w3_v[:, k * OUT:(k + 1) * OUT],
                        h2[:, k * NPIX + px0:k * NPIX + px0 + npx],
                        start=(k == 0), stop=(k == 1))
                nc.vector.tensor_scalar_add(
                    out_sb[:, px0:px0 + npx], ps[:], b3_v)

            def out_dma(px0, px1, last=False):
                eng = nc.sync if last else nc.scalar
                eng.dma_start(out[:, px0:px1], out_sb[:, px0:px1])

            def l1(px0, npx):
                emit_dense("l1", h0, h1, w1_v, b1_v, px0, npx)

            def l2(px0, npx):
                emit_dense("l2", h1, h2, w2_v, b2_v, px0, npx)

            for i in range(ROWS + 2):
                if i < ROWS:
                    emit_l0_unit(0, i)
                if i >= 2:
                    z = i - 2
                    emit_l0_unit(1, z)
                    if z % 4 == 3 and z < 28:
                        t = z // 4              # 0..6
                        l1(t * TP, TP)
                        if t >= 1:
                            l2((t - 1) * TP, TP)
                        if t >= 2:
                            emit_head((t - 2) * TP, TP)
                            if t in (3, 5, 7):
                                out_dma((t - 3) * TP, (t - 1) * TP)
                    elif z == 28:
                        l2(6 * TP, TP)
                    elif z == 29:
                        l1(3584, 256)           # tile 7 first half
                        emit_head(5 * TP, TP)
                        out_dma(4 * TP, 6 * TP)
                    elif z == 30:
                        l2(3584, 256)
                    elif z == 31:
                        emit_head(6 * TP, TP)
                        l1(3840, 256)
                        emit_head(3584, 256)
                        out_dma(6 * TP, 3840)
                        l2(3840, 256)
                        emit_head(3840, 256)
                        out_dma(3840, NPIX, last=True)

    nc.finalize()
    _BUILT[key] = nc
    return nc


def _to_f32r(a):
    """Round fp32 to the fp32r format the PE expects (low 12 mantissa bits 0)."""
    b = np.ascontiguousarray(a, np.float32).view(np.uint32).astype(np.uint64)
    r = ((b + 0x800) & 0xFFFFF000).astype(np.uint32)
    return r.view(np.float32).reshape(np.asarray(a).shape)


def _e4(a):
    return np.ascontiguousarray(a, np.float32).astype(E4)


def _prep_core_inputs(c, xi, gx, gy):
    b = c // 4
    y0 = (c % 4) * ROWS
    slab = np.zeros((128, SLABR, SLABW), np.float32)
    ylo, yhi = y0 - 2, y0 + ROWS + 2
    slo, shi = max(ylo, 0), min(yhi, H)
    slab[:, slo - ylo: shi - ylo, 2:2 + W] = xi[b, :, slo:shi, :]
    slab *= SX
    xh = _e4(slab)
    xl = _e4(slab - xh.astype(np.float32))

    csl = np.zeros((128, SLABR, SLABW), np.float32)
    csl[0, 2:35, 2:130] = SX * gx[None, :]
    # gy per slab row r (used at window row y'+2 -> image row y0+y'):
    for r in range(2, 35):
        csl[1, r, 2:130] = SX * gy[min(max(y0 + r - 2, 0), H - 1)]

    return {
        "xs": np.concatenate(
            [xh.reshape(128, SLAB), xl.reshape(128, SLAB),
             _e4(csl.reshape(128, SLAB))], axis=1),
    }


def kernel(**inputs):
    from concourse.bass_utils import run_bass_kernel_spmd

    xi = np.asarray(inputs["xi"], np.float32)
    W0 = np.asarray(inputs["W0"], np.float32)
    b0 = np.asarray(inputs["b0"], np.float32)
    W1 = np.asarray(inputs["W1"], np.float32)
    b1 = np.asarray(inputs["b1"], np.float32)
    W2 = np.asarray(inputs["W2"], np.float32)
    b2 = np.asarray(inputs["b2"], np.float32)
    W3 = np.asarray(inputs["W3"], np.float32)
    b3 = np.asarray(inputs["b3"], np.float32)

    # ---- weight prep (replicated) ----
    # patch rows of W0 are (c, dy, dx)-ordered; ktile k=(dy*5+dx) gathers
    # rows c*25+k. Scale 2^12, split hi/lo in e4m3.
    Wp = (SWT * W0[:FC * P * P]).reshape(128, 25, HID)   # [c, k, out]
    wh_f = _e4(Wp).astype(np.float32)
    wl_f = Wp - wh_f
    # coords weight pad: [m][2 halves][128]; half0 rows 0,1 = SWT*Wc
    Wc = SWT * W0[FC * P * P:]                            # [2, 256]
    wcp = np.zeros((128, 2, 2, 128), np.float32)
    for m in range(2):
        wcp[0:2, m, 0, :] = Wc[:, m * 128:(m + 1) * 128]
    # w0h: [wcp(512)][m=0 taps][m=1 taps]
    w0h_pk = np.empty((128, 512 + 6400), np.float32)
    w0h_pk[:, 0:512] = wcp.reshape(128, 512)
    for m in range(2):
        for k in range(25):
            off = 512 + m * 3200 + k * 128
            w0h_pk[:, off:off + 128] = wh_f[:, k, m * 128:(m + 1) * 128]
    # w0l: [b0(8 bytes)][m=0 pair blocks][m=1 pair blocks]
    b0_h = np.ascontiguousarray((OMEGA * b0).reshape(2, 128).T,
                                np.float32)               # [128, 2]
    w0l_pk = np.zeros((128, 8 + 2 * NC128), E4)
    w0l_pk[:, 0:8] = b0_h.view(np.uint8).reshape(128, 8).view(E4)
    for m in range(2):
        for pi, (dx, q) in enumerate(WPAIRS):
            for j in range(2):
                k = (2 * q + j) * 5 + dx
                off = 8 + NC128 * m + pi * 256 + j * 128
                w0l_pk[:, off:off + 128] = _e4(
                    wl_f[:, k, m * 128:(m + 1) * 128])

    # wtail: [w1|w2|w3|b1|b2|b3] as one f32(r) plane
    wt_pk = np.zeros((128, WTAIL), np.float32)
    wt_pk[:, 0:512] = _to_f32r(
        W1.reshape(2, 128, 2, 128).transpose(1, 0, 2, 3).reshape(128, 512))
    wt_pk[:, 512:1024] = _to_f32r(
        W2.reshape(2, 128, 2, 128).transpose(1, 0, 2, 3).reshape(128, 512))
    wt_pk[:, 1024:1030] = _to_f32r(
        W3.reshape(2, 128, OUT).transpose(1, 0, 2).reshape(128, 2 * OUT))
    wt_pk[:, 1030:1032] = np.ascontiguousarray((OMEGA * b1).reshape(2, 128).T)
    wt_pk[:, 1032:1034] = np.ascontiguousarray((OMEGA * b2).reshape(2, 128).T)
    wt_pk[0:OUT, 1034] = b3

    ys = np.linspace(-1.0, 1.0, H, dtype=np.float32)
    xcs = np.linspace(-1.0, 1.0, W, dtype=np.float32)

    shared = {"w0h": _e4(w0h_pk), "w0l": w0l_pk, "wt": wt_pk}
    in_maps = []
    for c in range(NCORES):
        m = _prep_core_inputs(c, xi, xcs, ys)
        m.update(shared)
        in_maps.append(m)

    nc = _build()
    res = run_bass_kernel_spmd(nc, in_maps, core_ids=list(range(NCORES)))
    global LAST_RES
    LAST_RES = res

    full = np.empty((B, OUT, H, W), np.float32)
    for c in range(NCORES):
        b = c // 4
        y0 = (c % 4) * ROWS
        full[b, :, y0:y0 + ROWS, :] = res.results[c]["out"].reshape(
            OUT, ROWS, W)
    return full


# revision 14
# speedup vs baseline: 1.0950x; 1.0220x over previous
"""NeRD pixel decoder (SIREN MLP over 5x5 local patches) on 8 trn2 cores.

Sharding: row-shard the pixel dim. Core c handles image b=c//4, rows
y0=(c%4)*32 .. y0+32 (4096 pixels). SIREN weights replicated.

Layer 0 (the 5x5 conv, 84% of FLOPs) runs in fp8-e4m3 DoubleRow matmuls at
0.5 cycles/row: per output row and 128-out-chan block, 25 taps are computed
as DR pairs (x_hi, x_lo) against stride-0-duplicated fp8 weights (x split
into hi + lo e4m3 parts on host, recovering ~11-bit input precision), plus a
weight-residual correction pass (w_lo pairs over vertically adjacent taps,
whose windows don't overlap -- overlapping DR rhs windows crash the PE) for
the first NCORR taps, plus a coords pair (gx/gy baked into a slab-shaped fp8
plane, phantom zero-weight second half). Layers 1/2 and the head stay in
float32r at full PE rate.

v8 pipeline: ten input DMAs total (hi/lo/coords slabs ride one tensor in
row chunks; wcp+b0 ride the w0 tensors via byte-packing; w1/w2/w3/b1/b2/b3
ride one f32r "wtail" tensor with bitcast bias slices), split across both
HWDGE queues in deadline order -- each dma_start costs ~650ns of sequencer
+ HWDGE issue, which dominates the lead-in, so fewer is faster. Dummy fp8
warmup matmuls cover the DMA lead-in and the PE p-state ramp. m=1 row units
lag m=0 by two rows so the m=1 weight DMA can land later. L1/L2/head tiles
are fused into the L0 row stream with a one-tile stagger (and a split final
tile) so cross-engine dependencies are old when the PE reaches them and the
drain tail is short. Per-tile output DMA.

Everything is quantized host-side (e4m3 via ml_dtypes, f32r by mantissa
rounding); the device only multiplies exactly and accumulates in f32 PSUM.
Weight scale 2^12 and x scale 2^2 keep e4m3 operands in normal range; the
activation scale folds 2^-14 back out (sin(OMEGA*(z+b0)) via ACT bias).
"""

import numpy as np
import ml_dtypes

FC = 128      # feature channels
P = 5         # patch
HID = 256
OUT = 3
OMEGA = 30.0
B, H, W = 2, 128, 128
NCORES = 8
ROWS = H // 4            # 32 image rows per core
NPIX = ROWS * W          # 4096 pixels per core
SLABR = ROWS + 4         # 36 slab rows (2 halo each side)
SLABW = W + 4            # 132 slab cols (2 pad each side)
SLAB = SLABR * SLABW     # 4752
TP = 512                 # pixels per L1/L2/head PSUM tile (= 4 image rows)
NT = NPIX // TP          # 8 tiles per core

E4 = ml_dtypes.float8_e4m3
SX = 4.0                 # x (slab/coords) pre-scale
SWT = 4096.0             # weight pre-scale
NCORR = 15               # w_lo-corrected taps: 7 vertical pairs + tap12 in the
                         # mixed pair whose second half is the coords plane
NWARM = 50               # warmup DR matmuls during DMA lead-in
WPAIRS = [(dx, q) for q in range(2) for dx in range(5)][:7]  # 14 taps; +tap12 mixed
WLBLK = 8 * 256          # per-m w0l bytes: 8 DR pair blocks
WTAIL = 1036             # packed w1|w2|w3|b1|b2|b3 columns (f32 each)

_BUILT = {}


def _build(structure="v8"):
    key = structure
    if key in _BUILT:
        return _BUILT[key]

    import concourse.tile as tile
    import concourse.mybir as mybir
    from concourse import bacc

    f32 = mybir.dt.float32
    f32r = mybir.dt.float32r
    fp8 = mybir.dt.float8e4
    Sin = mybir.ActivationFunctionType.Sin
    DR = mybir.MatmulPerfMode.DoubleRow

    nc = bacc.Bacc("TRN2", target_bir_lowering=False, debug=False)

    xs = nc.dram_tensor("xs", [128, 3 * SLAB], fp8, kind="ExternalInput").ap()
    w0h = nc.dram_tensor("w0h", [128, 6400], fp8,
                         kind="ExternalInput").ap()
    w0l = nc.dram_tensor("w0l", [128, 8 + 2 * WLBLK], fp8,
                         kind="ExternalInput").ap()
    wt = nc.dram_tensor("wt", [128, WTAIL], f32r, kind="ExternalInput").ap()
    out = nc.dram_tensor("out", [OUT, NPIX], f32, kind="ExternalOutput").ap()

    with tile.TileContext(nc) as tc:
        with (
            tc.tile_pool(name="const", bufs=1) as cpool,
            tc.tile_pool(name="h", bufs=3) as hpool,
            tc.tile_pool(name="osb", bufs=1) as opool,
            tc.tile_pool(name="ps", bufs=8, space="PSUM") as pspool,
        ):
            # ---- SBUF tiles ----
            xs_t = cpool.tile([128, 3 * SLAB], fp8, tag="xs", name="xs_t")
            w0h_t = cpool.tile([128, 6400], fp8, tag="w0h", name="w0h_t")
            w0l_t = cpool.tile([128, 8 + 2 * WLBLK], fp8, tag="w0l",
                               name="w0l_t")
            wt_t = cpool.tile([128, WTAIL], f32r, tag="wt", name="wt_t")
            scr8 = cpool.tile([128, 256], fp8, tag="scr8", name="scr8")
            scro = cpool.tile([128, 128], f32, tag="scro", name="scro")
            out_sb = opool.tile([OUT, NPIX], f32, tag="osb")

            # packed views
            b0_v = w0l_t[:, 0:8].bitcast(f32)          # [128, 2]
            w1_v = wt_t[:, 0:512]
            w2_v = wt_t[:, 512:1024]
            w3_v = wt_t[:, 1024:1030]
            b1_v = wt_t[:, 1030:1032].bitcast(f32)
            b2_v = wt_t[:, 1032:1034].bitcast(f32)
            b3_v = wt_t[:][0:OUT, 1034:1035].bitcast(f32)   # [3, 1]

            xs3 = xs.rearrange("p (s n) -> p s n", s=3)
            xst3 = xs_t[:].rearrange("p (s n) -> p s n", s=3)

            def slab_rows(ap3, r0, r1):
                return ap3[:, :, r0 * SLABW:r1 * SLABW]

            # ---- input DMAs: deadline order, two HWDGE queues ----
            nc.sync.dma_start(slab_rows(xst3, 0, 8), slab_rows(xs3, 0, 8))
            nc.sync.dma_start(w0h_t[:, 0:3200], w0h[:, 0:3200])  # m0
            nc.sync.dma_start(w0l_t[:, 0:8 + WLBLK],
                              w0l[:, 0:8 + WLBLK])               # b0 + m0
            nc.sync.dma_start(w0h_t[:, 3200:6400], w0h[:, 3200:6400])  # m1
            nc.sync.dma_start(w0l_t[:, 8 + WLBLK:], w0l[:, 8 + WLBLK:])
            nc.sync.dma_start(slab_rows(xst3, 8, 16), slab_rows(xs3, 8, 16))
            nc.sync.dma_start(slab_rows(xst3, 16, 24), slab_rows(xs3, 16, 24))
            nc.sync.dma_start(slab_rows(xst3, 24, 32), slab_rows(xs3, 24, 32))
            nc.sync.dma_start(slab_rows(xst3, 32, 36), slab_rows(xs3, 32, 36))
            nc.sync.dma_start(wt_t[:], wt[:])

            # ---- PE warmup on scratch data (p-state ramp during DMA) ----
            nc.vector.memset(scr8[:], 0.0)
            scr3 = scr8[:].rearrange("p (s n) -> p s n", s=2)
            for i in range(NWARM):
                psw = pspool.tile([128, 128], f32, tag="ps", name=f"psw{i}")
                nc.tensor.matmul(psw[:], scr3, scr3, start=True, stop=True,
                                 perf_mode=DR)
                if i == NWARM - 1:
                    nc.scalar.activation(scro[:], psw[:], Sin,
                                         bias=b0_v[:, 0:1], scale=1.0)

            # ---- fused pipeline ----
            h0 = hpool.tile([128, 2 * NPIX], f32r, tag="h", name="h0")
            h1 = hpool.tile([128, 2 * NPIX], f32r, tag="h", name="h1")
            h2 = hpool.tile([128, 2 * NPIX], f32r, tag="h", name="h2")
            act_scale = OMEGA / (SX * SWT)

            def w0h_blk(m, k):
                off = m * 3200 + k * 128
                a = w0h_t[:, off:off + 128].unsqueeze(1).copy()
                a.ap[1] = [0, 2]   # stride-0: same hi-weights for both halves
                return a

            def emit_l0_unit(m, y):
                ps = pspool.tile([128, 128], f32, tag="ps",
                                 name=f"ps_l0_{m}_{y}")
                for k in range(25):
                    dy, dx = divmod(k, 5)
                    off = (y + dy) * SLABW + dx
                    nc.tensor.matmul(ps[:], w0h_blk(m, k),
                                     xst3[:, 0:2, off:off + 128],
                                     start=(k == 0), stop=False, perf_mode=DR)
                for pi, (dx, q) in enumerate(WPAIRS):
                    blk = 8 + WLBLK * m + pi * 256
                    lhs = w0l_t[:, blk:blk + 256].rearrange(
                        "p (t c) -> p t c", t=2)
                    off = (y + 2 * q) * SLABW + dx
                    rhs = xst3[:, 0:1, off:off + 128].copy()
                    rhs.ap[1] = [SLABW, 2]       # taps (2q,dx), (2q+1,dx)
                    nc.tensor.matmul(ps[:], lhs, rhs, start=False,
                                     stop=False, perf_mode=DR)
                # mixed pair last: (w_lo of tap12) x window + wcp x coords
                # plane -- tap12's window offset equals the coords window's
                # in-plane offset, so the two-dim stride is exactly 2*SLAB
                blk = 8 + WLBLK * m + 7 * 256
                lhs = w0l_t[:, blk:blk + 256].rearrange(
                    "p (t c) -> p t c", t=2)
                off = (y + 2) * SLABW + 2
                rhs = xst3[:, 0:1, off:off + 128].copy()
                rhs.ap[1] = [2 * SLAB, 2]
                nc.tensor.matmul(ps[:], lhs, rhs, start=False, stop=True,
                                 perf_mode=DR)
                nc.scalar.activation(
                    h0[:, m * NPIX + y * 128:m * NPIX + (y + 1) * 128],
                    ps[:], Sin, bias=b0_v[:, m:m + 1], scale=act_scale)

            def emit_dense(lname, hin, hout, wl_v, bl_v, px0, npx):
                for m in range(2):
                    ps = pspool.tile([128, npx], f32, tag="ps",
                                     name=f"ps_{lname}_{m}_{px0}")
                    for k in range(2):
                        nc.tensor.matmul(
                            ps[:],
                            wl_v[:, (k * 2 + m) * 128:(k * 2 + m + 1) * 128],
                            hin[:, k * NPIX + px0:k * NPIX + px0 + npx],
                            start=(k == 0), stop=(k == 1))
                    nc.scalar.activation(
                        hout[:, m * NPIX + px0:m * NPIX + px0 + npx], ps[:],
                        Sin, bias=bl_v[:, m:m + 1], scale=OMEGA)

            def emit_head(px0, npx):
                ps = pspool.tile([OUT, npx], f32, tag="ps",
                                 name=f"ps_hd_{px0}")
                for k in range(2):
                    nc.tensor.matmul(
                        ps[:], w3_v[:, k * OUT:(k + 1) * OUT],
                        h2[:, k * NPIX + px0:k * NPIX + px0 + npx],
                        start=(k == 0), stop=(k == 1))
                nc.vector.tensor_scalar_add(
                    out_sb[:, px0:px0 + npx], ps[:], b3_v)

            def out_dma(px0, px1, last=False):
                eng = nc.sync if last else nc.scalar
                eng.dma_start(out[:, px0:px1], out_sb[:, px0:px1])

            def l1(px0, npx):
                emit_dense("l1", h0, h1, w1_v, b1_v, px0, npx)

            def l2(px0, npx):
                emit_dense("l2", h1, h2, w2_v, b2_v, px0, npx)

            for i in range(ROWS + 2):
                if i < ROWS:
                    emit_l0_unit(0, i)
                if i >= 2:
                    z = i - 2
                    emit_l0_unit(1, z)
                    if z % 4 == 3 and z < 28:
                        t = z // 4              # 0..6
                        l1(t * TP, TP)
                        if t >= 1:
                            l2((t - 1) * TP, TP)
                        if t >= 2:
                            emit_head((t - 2) * TP, TP)
                            if t in (3, 5, 7):
                                out_dma((t - 3) * TP, (t - 1) * TP)
                    elif z == 28:
                        l2(6 * TP, TP)
                    elif z == 29:
                        l1(3584, 256)           # tile 7 first half
                        emit_head(5 * TP, TP)
                        out_dma(4 * TP, 6 * TP)
                    elif z == 30:
                        l2(3584, 256)
                    elif z == 31:
                        emit_head(6 * TP, TP)
                        emit_head(3584, 256)
                        out_dma(6 * TP, 3840)
                        l1(3840, 256)
                        l2(3840, 256)
                        emit_head(3840, 256)
                        out_dma(3840, NPIX, last=True)

    nc.finalize()
    _BUILT[key] = nc
    return nc


def _to_f32r(a):
    """Round fp32 to the fp32r format the PE expects (low 12 mantissa bits 0)."""
    b = np.ascontiguousarray(a, np.float32).view(np.uint32).astype(np.uint64)
    r = ((b + 0x800) & 0xFFFFF000).astype(np.uint32)
    return r.view(np.float32).reshape(np.asarray(a).shape)


def _e4(a):
    return np.ascontiguousarray(a, np.float32).astype(E4)


def _prep_core_inputs(c, xi, gx, gy):
    b = c // 4
    y0 = (c % 4) * ROWS
    slab = np.zeros((128, SLABR, SLABW), np.float32)
    ylo, yhi = y0 - 2, y0 + ROWS + 2
    slo, shi = max(ylo, 0), min(yhi, H)
    slab[:, slo - ylo: shi - ylo, 2:2 + W] = xi[b, :, slo:shi, :]
    slab *= SX
    xh = _e4(slab)
    xl = _e4(slab - xh.astype(np.float32))

    csl = np.zeros((128, SLABR, SLABW), np.float32)
    csl[0, 2:35, 2:130] = SX * gx[None, :]
    # gy per slab row r (used at window row y'+2 -> image row y0+y'):
    for r in range(2, 35):
        csl[1, r, 2:130] = SX * gy[min(max(y0 + r - 2, 0), H - 1)]

    return {
        "xs": np.concatenate(
            [xh.reshape(128, SLAB), xl.reshape(128, SLAB),
             _e4(csl.reshape(128, SLAB))], axis=1),
    }


def kernel(**inputs):
    from concourse.bass_utils import run_bass_kernel_spmd

    xi = np.asarray(inputs["xi"], np.float32)
    W0 = np.asarray(inputs["W0"], np.float32)
    b0 = np.asarray(inputs["b0"], np.float32)
    W1 = np.asarray(inputs["W1"], np.float32)
    b1 = np.asarray(inputs["b1"], np.float32)
    W2 = np.asarray(inputs["W2"], np.float32)
    b2 = np.asarray(inputs["b2"], np.float32)
    W3 = np.asarray(inputs["W3"], np.float32)
    b3 = np.asarray(inputs["b3"], np.float32)

    # ---- weight prep (replicated) ----
    # patch rows of W0 are (c, dy, dx)-ordered; ktile k=(dy*5+dx) gathers
    # rows c*25+k. Scale 2^12, split hi/lo in e4m3.
    Wp = (SWT * W0[:FC * P * P]).reshape(128, 25, HID)   # [c, k, out]
    wh_f = _e4(Wp).astype(np.float32)
    wl_f = Wp - wh_f
    # coords weight pad: [m][2 halves][128]; half0 rows 0,1 = SWT*Wc
    Wc = SWT * W0[FC * P * P:]                            # [2, 256]
    wcp = np.zeros((128, 2, 2, 128), np.float32)
    for m in range(2):
        wcp[0:2, m, 0, :] = Wc[:, m * 128:(m + 1) * 128]
    # w0h: [m=0 taps][m=1 taps]
    w0h_pk = np.empty((128, 6400), np.float32)
    for m in range(2):
        for k in range(25):
            off = m * 3200 + k * 128
            w0h_pk[:, off:off + 128] = wh_f[:, k, m * 128:(m + 1) * 128]
    # w0l: [b0(8 bytes)][m=0 pair blocks][m=1 pair blocks]
    b0_h = np.ascontiguousarray((OMEGA * b0).reshape(2, 128).T,
                                np.float32)               # [128, 2]
    w0l_pk = np.zeros((128, 8 + 2 * WLBLK), E4)
    w0l_pk[:, 0:8] = b0_h.view(np.uint8).reshape(128, 8).view(E4)
    for m in range(2):
        for pi, (dx, q) in enumerate(WPAIRS):
            for j in range(2):
                k = (2 * q + j) * 5 + dx
                off = 8 + WLBLK * m + pi * 256 + j * 128
                w0l_pk[:, off:off + 128] = _e4(
                    wl_f[:, k, m * 128:(m + 1) * 128])
        off = 8 + WLBLK * m + 7 * 256
        w0l_pk[:, off:off + 128] = _e4(wl_f[:, 12, m * 128:(m + 1) * 128])
        w0l_pk[:, off + 128:off + 256] = _e4(wcp[:, m, 0, :])

    # wtail: [w1|w2|w3|b1|b2|b3] as one f32(r) plane
    wt_pk = np.zeros((128, WTAIL), np.float32)
    wt_pk[:, 0:512] = _to_f32r(
        W1.reshape(2, 128, 2, 128).transpose(1, 0, 2, 3).reshape(128, 512))
    wt_pk[:, 512:1024] = _to_f32r(
        W2.reshape(2, 128, 2, 128).transpose(1, 0, 2, 3).reshape(128, 512))
    wt_pk[:, 1024:1030] = _to_f32r(
        W3.reshape(2, 128, OUT).transpose(1, 0, 2).reshape(128, 2 * OUT))
    wt_pk[:, 1030:1032] = np.ascontiguousarray((OMEGA * b1).reshape(2, 128).T)
    wt_pk[:, 1032:1034] = np.ascontiguousarray((OMEGA * b2).reshape(2, 128).T)
    wt_pk[0:OUT, 1034] = b3

    ys = np.linspace(-1.0, 1.0, H, dtype=np.float32)
    xcs = np.linspace(-1.0, 1.0, W, dtype=np.float32)

    shared = {"w0h": _e4(w0h_pk), "w0l": w0l_pk, "wt": wt_pk}
    in_maps = []
    for c in range(NCORES):
        m = _prep_core_inputs(c, xi, xcs, ys)
        m.update(shared)
        in_maps.append(m)

    nc = _build()
    res = run_bass_kernel_spmd(nc, in_maps, core_ids=list(range(NCORES)))
    global LAST_RES
    LAST_RES = res

    full = np.empty((B, OUT, H, W), np.float32)
    for c in range(NCORES):
        b = c // 4
        y0 = (c % 4) * ROWS
        full[b, :, y0:y0 + ROWS, :] = res.results[c]["out"].reshape(
            OUT, ROWS, W)
    return full


# revision 18
# speedup vs baseline: 1.1090x; 1.0128x over previous
"""NeRD pixel decoder (SIREN MLP over 5x5 local patches) on 8 trn2 cores.

Sharding: row-shard the pixel dim. Core c handles image b=c//4, rows
y0=(c%4)*32 .. y0+32 (4096 pixels). SIREN weights replicated.

Layer 0 (the 5x5 conv, 84% of FLOPs) runs in fp8-e4m3 DoubleRow matmuls at
0.5 cycles/row: per output row and 128-out-chan block, 25 taps are computed
as DR pairs (x_hi, x_lo) against stride-0-duplicated fp8 weights (x split
into hi + lo e4m3 parts on host, recovering ~11-bit input precision), plus a
weight-residual correction pass (w_lo pairs over vertically adjacent taps,
whose windows don't overlap -- overlapping DR rhs windows crash the PE) for
the first NCORR taps, plus a coords pair (gx/gy baked into a slab-shaped fp8
plane, phantom zero-weight second half). Layers 1/2 and the head stay in
float32r at full PE rate.

v8 pipeline: ten input DMAs total (hi/lo/coords slabs ride one tensor in
row chunks; wcp+b0 ride the w0 tensors via byte-packing; w1/w2/w3/b1/b2/b3
ride one f32r "wtail" tensor with bitcast bias slices), split across both
HWDGE queues in deadline order -- each dma_start costs ~650ns of sequencer
+ HWDGE issue, which dominates the lead-in, so fewer is faster. Dummy fp8
warmup matmuls cover the DMA lead-in and the PE p-state ramp. m=1 row units
lag m=0 by two rows so the m=1 weight DMA can land later. L1/L2/head tiles
are fused into the L0 row stream with a one-tile stagger (and a split final
tile) so cross-engine dependencies are old when the PE reaches them and the
drain tail is short. Per-tile output DMA.

Everything is quantized host-side (e4m3 via ml_dtypes, f32r by mantissa
rounding); the device only multiplies exactly and accumulates in f32 PSUM.
Weight scale 2^12 and x scale 2^2 keep e4m3 operands in normal range; the
activation scale folds 2^-14 back out (sin(OMEGA*(z+b0)) via ACT bias).
"""

import numpy as np
import ml_dtypes

FC = 128      # feature channels
P = 5         # patch
HID = 256
OUT = 3
OMEGA = 30.0
B, H, W = 2, 128, 128
NCORES = 8
ROWS = H // 4            # 32 image rows per core
NPIX = ROWS * W          # 4096 pixels per core
SLABR = ROWS + 4         # 36 slab rows (2 halo each side)
SLABW = W + 4            # 132 slab cols (2 pad each side)
SLAB = SLABR * SLABW     # 4752
TP = 512                 # pixels per L1/L2/head PSUM tile (= 4 image rows)
NT = NPIX // TP          # 8 tiles per core

E4 = ml_dtypes.float8_e4m3
SX = 4.0                 # x (slab/coords) pre-scale
SWT = 4096.0             # weight pre-scale
NCORR = 15               # w_lo-corrected taps: 7 vertical pairs + tap12 in the
                         # mixed pair whose second half is the coords plane
NWARM = 84               # warmup DR matmuls during DMA lead-in
WPAIRS = [(dx, q) for q in range(2) for dx in range(5)][:7]  # 14 taps; +tap12 mixed
WLBLK = 8 * 256          # per-m w0l bytes: 8 DR pair blocks
WTAIL = 5                # packed b1|b2|b3 columns (f32)
WBC = 1030               # packed w1|w2|w3 columns (bf16)

_BUILT = {}


def _build(structure="v8"):
    key = structure
    if key in _BUILT:
        return _BUILT[key]

    import concourse.tile as tile
    import concourse.mybir as mybir
    from concourse import bacc

    f32 = mybir.dt.float32
    f32r = mybir.dt.float32r
    fp8 = mybir.dt.float8e4
    Sin = mybir.ActivationFunctionType.Sin
    DR = mybir.MatmulPerfMode.DoubleRow

    nc = bacc.Bacc("TRN2", target_bir_lowering=False, debug=False)

    xs = nc.dram_tensor("xs", [128, 3 * SLAB], fp8, kind="ExternalInput").ap()
    w0h = nc.dram_tensor("w0h", [128, 6400], fp8,
                         kind="ExternalInput").ap()
    w0l = nc.dram_tensor("w0l", [128, 8 + 2 * WLBLK], fp8,
                         kind="ExternalInput").ap()
    wt = nc.dram_tensor("wt", [128, WTAIL], f32, kind="ExternalInput").ap()
    wb = nc.dram_tensor("wb", [128, WBC], mybir.dt.bfloat16,
                        kind="ExternalInput").ap()
    out = nc.dram_tensor("out", [OUT, NPIX], f32, kind="ExternalOutput").ap()

    with tile.TileContext(nc) as tc:
        with (
            tc.tile_pool(name="const", bufs=1) as cpool,
            tc.tile_pool(name="h", bufs=3) as hpool,
            tc.tile_pool(name="osb", bufs=1) as opool,
            tc.tile_pool(name="ps", bufs=8, space="PSUM") as pspool,
        ):
            # ---- SBUF tiles ----
            xs_t = cpool.tile([128, 3 * SLAB], fp8, tag="xs", name="xs_t")
            w0h_t = cpool.tile([128, 6400], fp8, tag="w0h", name="w0h_t")
            w0l_t = cpool.tile([128, 8 + 2 * WLBLK], fp8, tag="w0l",
                               name="w0l_t")
            wt_t = cpool.tile([128, WTAIL], f32, tag="wt", name="wt_t")
            wb_t = cpool.tile([128, WBC], mybir.dt.bfloat16, tag="wb",
                              name="wb_t")
            scr8 = cpool.tile([128, 256], fp8, tag="scr8", name="scr8")
            scro = cpool.tile([128, 128], f32, tag="scro", name="scro")
            out_sb = opool.tile([OUT, NPIX], f32, tag="osb")

            # packed views
            b0_v = w0l_t[:, 0:8].bitcast(f32)          # [128, 2]
            w1_v = wb_t[:, 0:512]
            w2_v = wb_t[:, 512:1024]
            w3_v = wb_t[:, 1024:1030]
            b1_v = wt_t[:, 0:2]
            b2_v = wt_t[:, 2:4]
            b3_v = wt_t[:][0:OUT, 4:5]                 # [3, 1]

            xs3 = xs.rearrange("p (s n) -> p s n", s=3)
            xst3 = xs_t[:].rearrange("p (s n) -> p s n", s=3)

            def slab_rows(ap3, r0, r1):
                return ap3[:, :, r0 * SLABW:r1 * SLABW]

            # ---- input DMAs: deadline order, two HWDGE queues ----
            nc.sync.dma_start(slab_rows(xst3, 0, 6), slab_rows(xs3, 0, 6))
            nc.sync.dma_start(w0h_t[:, 0:3200], w0h[:, 0:3200])  # m0
            nc.sync.dma_start(w0l_t[:, 0:8 + WLBLK],
                              w0l[:, 0:8 + WLBLK])               # b0 + m0
            nc.sync.dma_start(w0h_t[:, 3200:6400], w0h[:, 3200:6400])  # m1
            nc.sync.dma_start(w0l_t[:, 8 + WLBLK:], w0l[:, 8 + WLBLK:])
            nc.sync.dma_start(slab_rows(xst3, 6, 14), slab_rows(xs3, 6, 14))
            nc.sync.dma_start(slab_rows(xst3, 14, 22), slab_rows(xs3, 14, 22))
            nc.sync.dma_start(slab_rows(xst3, 22, 30), slab_rows(xs3, 22, 30))
            nc.sync.dma_start(slab_rows(xst3, 30, 36), slab_rows(xs3, 30, 36))
            nc.sync.dma_start(wb_t[:], wb[:])
            nc.sync.dma_start(wt_t[:], wt[:])

            # ---- PE warmup on scratch data (p-state ramp during DMA) ----
            nc.vector.memset(scr8[:], 0.0)
            scr3 = scr8[:].rearrange("p (s n) -> p s n", s=2)
            for i in range(NWARM):
                psw = pspool.tile([128, 128], f32, tag="ps", name=f"psw{i}")
                nc.tensor.matmul(psw[:], scr3, scr3, start=True, stop=True,
                                 perf_mode=DR)
                if i == NWARM - 1:
                    nc.scalar.activation(scro[:], psw[:], Sin,
                                         bias=b0_v[:, 0:1], scale=1.0)

            # ---- fused pipeline ----
            bf16 = mybir.dt.bfloat16
            h0 = hpool.tile([128, 2 * NPIX], bf16, tag="h", name="h0")
            h1 = hpool.tile([128, 2 * NPIX], bf16, tag="h", name="h1")
            h2 = hpool.tile([128, 2 * NPIX], bf16, tag="h", name="h2")
            act_scale = OMEGA / (SX * SWT)

            def w0h_blk(m, k):
                off = m * 3200 + k * 128
                a = w0h_t[:, off:off + 128].unsqueeze(1).copy()
                a.ap[1] = [0, 2]   # stride-0: same hi-weights for both halves
                return a

            def emit_l0_unit(m, y):
                ps = pspool.tile([128, 128], f32, tag="ps",
                                 name=f"ps_l0_{m}_{y}")
                for k in range(25):
                    dy, dx = divmod(k, 5)
                    off = (y + dy) * SLABW + dx
                    nc.tensor.matmul(ps[:], w0h_blk(m, k),
                                     xst3[:, 0:2, off:off + 128],
                                     start=(k == 0), stop=False, perf_mode=DR)
                for pi, (dx, q) in enumerate(WPAIRS):
                    blk = 8 + WLBLK * m + pi * 256
                    lhs = w0l_t[:, blk:blk + 256].rearrange(
                        "p (t c) -> p t c", t=2)
                    off = (y + 2 * q) * SLABW + dx
                    rhs = xst3[:, 0:1, off:off + 128].copy()
                    rhs.ap[1] = [SLABW, 2]       # taps (2q,dx), (2q+1,dx)
                    nc.tensor.matmul(ps[:], lhs, rhs, start=False,
                                     stop=False, perf_mode=DR)
                # mixed pair last: (w_lo of tap12) x window + wcp x coords
                # plane -- tap12's window offset equals the coords window's
                # in-plane offset, so the two-dim stride is exactly 2*SLAB
                blk = 8 + WLBLK * m + 7 * 256
                lhs = w0l_t[:, blk:blk + 256].rearrange(
                    "p (t c) -> p t c", t=2)
                off = (y + 2) * SLABW + 2
                rhs = xst3[:, 0:1, off:off + 128].copy()
                rhs.ap[1] = [2 * SLAB, 2]
                nc.tensor.matmul(ps[:], lhs, rhs, start=False, stop=True,
                                 perf_mode=DR)
                nc.scalar.activation(
                    h0[:, m * NPIX + y * 128:m * NPIX + (y + 1) * 128],
                    ps[:], Sin, bias=b0_v[:, m:m + 1], scale=act_scale)

            def emit_dense(lname, hin, hout, wl_v, bl_v, px0, npx):
                for m in range(2):
                    ps = pspool.tile([128, npx], f32, tag="ps",
                                     name=f"ps_{lname}_{m}_{px0}")
                    for k in range(2):
                        nc.tensor.matmul(
                            ps[:],
                            wl_v[:, (k * 2 + m) * 128:(k * 2 + m + 1) * 128],
                            hin[:, k * NPIX + px0:k * NPIX + px0 + npx],
                            start=(k == 0), stop=(k == 1))
                    nc.scalar.activation(
                        hout[:, m * NPIX + px0:m * NPIX + px0 + npx], ps[:],
                        Sin, bias=bl_v[:, m:m + 1], scale=OMEGA)

            def emit_head(px0, npx):
                ps = pspool.tile([OUT, npx], f32, tag="ps",
                                 name=f"ps_hd_{px0}")
                for k in range(2):
                    nc.tensor.matmul(
                        ps[:], w3_v[:, k * OUT:(k + 1) * OUT],
                        h2[:, k * NPIX + px0:k * NPIX + px0 + npx],
                        start=(k == 0), stop=(k == 1))
                nc.vector.tensor_scalar_add(
                    out_sb[:, px0:px0 + npx], ps[:], b3_v)

            def out_dma(px0, px1, last=False):
                nc.sync.dma_start(out[:, px0:px1], out_sb[:, px0:px1])

            def l1(px0, npx):
                emit_dense("l1", h0, h1, w1_v, b1_v, px0, npx)

            def l2(px0, npx):
                emit_dense("l2", h1, h2, w2_v, b2_v, px0, npx)

            for i in range(ROWS + 2):
                if i < ROWS:
                    emit_l0_unit(0, i)
                if i >= 2:
                    z = i - 2
                    emit_l0_unit(1, z)
                    if z % 4 == 3 and z < 28:
                        t = z // 4              # 0..6
                        l1(t * TP, TP)
                        if t >= 1:
                            l2((t - 1) * TP, TP)
                        if t >= 2:
                            emit_head((t - 2) * TP, TP)
                            if t in (3, 5, 7):
                                out_dma((t - 3) * TP, (t - 1) * TP)
                    elif z == 28:
                        l2(6 * TP, TP)
                    elif z == 29:
                        emit_head(5 * TP, TP)
                        out_dma(4 * TP, 6 * TP)
                        l1(3584, 256)           # tile 7 first half
                    elif z == 30:
                        l2(3584, 256)
                        l1(3840, 128)
                    elif z == 31:
                        emit_head(6 * TP, TP)
                        l2(3840, 128)
                        emit_head(3584, 256)
                        l1(3968, 128)
                        emit_head(3840, 128)
                        out_dma(6 * TP, 3968)
                        l2(3968, 128)
                        emit_head(3968, 128)
                        out_dma(3968, NPIX, last=True)

    nc.finalize()
    _BUILT[key] = nc
    return nc


def _to_f32r(a):
    """Round fp32 to the fp32r format the PE expects (low 12 mantissa bits 0)."""
    b = np.ascontiguousarray(a, np.float32).view(np.uint32).astype(np.uint64)
    r = ((b + 0x800) & 0xFFFFF000).astype(np.uint32)
    return r.view(np.float32).reshape(np.asarray(a).shape)


def _e4(a):
    return np.ascontiguousarray(a, np.float32).astype(E4)


def _prep_core_inputs(c, xi, gx, gy):
    b = c // 4
    y0 = (c % 4) * ROWS
    slab = np.zeros((128, SLABR, SLABW), np.float32)
    ylo, yhi = y0 - 2, y0 + ROWS + 2
    slo, shi = max(ylo, 0), min(yhi, H)
    slab[:, slo - ylo: shi - ylo, 2:2 + W] = xi[b, :, slo:shi, :]
    slab *= SX
    xh = _e4(slab)
    xl = _e4(slab - xh.astype(np.float32))

    csl = np.zeros((128, SLABR, SLABW), np.float32)
    csl[0, 2:35, 2:130] = SX * gx[None, :]
    # gy per slab row r (used at window row y'+2 -> image row y0+y'):
    for r in range(2, 35):
        csl[1, r, 2:130] = SX * gy[min(max(y0 + r - 2, 0), H - 1)]

    return {
        "xs": np.concatenate(
            [xh.reshape(128, SLAB), xl.reshape(128, SLAB),
             _e4(csl.reshape(128, SLAB))], axis=1),
    }


def kernel(**inputs):
    from concourse.bass_utils import run_bass_kernel_spmd

    xi = np.asarray(inputs["xi"], np.float32)
    W0 = np.asarray(inputs["W0"], np.float32)
    b0 = np.asarray(inputs["b0"], np.float32)
    W1 = np.asarray(inputs["W1"], np.float32)
    b1 = np.asarray(inputs["b1"], np.float32)
    W2 = np.asarray(inputs["W2"], np.float32)
    b2 = np.asarray(inputs["b2"], np.float32)
    W3 = np.asarray(inputs["W3"], np.float32)
    b3 = np.asarray(inputs["b3"], np.float32)

    # ---- weight prep (replicated) ----
    # patch rows of W0 are (c, dy, dx)-ordered; ktile k=(dy*5+dx) gathers
    # rows c*25+k. Scale 2^12, split hi/lo in e4m3.
    Wp = (SWT * W0[:FC * P * P]).reshape(128, 25, HID)   # [c, k, out]
    wh_f = _e4(Wp).astype(np.float32)
    wl_f = Wp - wh_f
    # coords weight pad: [m][2 halves][128]; half0 rows 0,1 = SWT*Wc
    Wc = SWT * W0[FC * P * P:]                            # [2, 256]
    wcp = np.zeros((128, 2, 2, 128), np.float32)
    for m in range(2):
        wcp[0:2, m, 0, :] = Wc[:, m * 128:(m + 1) * 128]
    # w0h: [m=0 taps][m=1 taps]
    w0h_pk = np.empty((128, 6400), np.float32)
    for m in range(2):
        for k in range(25):
            off = m * 3200 + k * 128
            w0h_pk[:, off:off + 128] = wh_f[:, k, m * 128:(m + 1) * 128]
    # w0l: [b0(8 bytes)][m=0 pair blocks][m=1 pair blocks]
    b0_h = np.ascontiguousarray((OMEGA * b0).reshape(2, 128).T,
                                np.float32)               # [128, 2]
    w0l_pk = np.zeros((128, 8 + 2 * WLBLK), E4)
    w0l_pk[:, 0:8] = b0_h.view(np.uint8).reshape(128, 8).view(E4)
    for m in range(2):
        for pi, (dx, q) in enumerate(WPAIRS):
            for j in range(2):
                k = (2 * q + j) * 5 + dx
                off = 8 + WLBLK * m + pi * 256 + j * 128
                w0l_pk[:, off:off + 128] = _e4(
                    wl_f[:, k, m * 128:(m + 1) * 128])
        off = 8 + WLBLK * m + 7 * 256
        w0l_pk[:, off:off + 128] = _e4(wl_f[:, 12, m * 128:(m + 1) * 128])
        w0l_pk[:, off + 128:off + 256] = _e4(wcp[:, m, 0, :])

    # wb: [w1|w2|w3] bf16; wt: [b1|b2|b3] f32
    wb_pk = np.zeros((128, WBC), ml_dtypes.bfloat16)
    wb_pk[:, 0:512] = W1.reshape(2, 128, 2, 128).transpose(
        1, 0, 2, 3).reshape(128, 512).astype(ml_dtypes.bfloat16)
    wb_pk[:, 512:1024] = W2.reshape(2, 128, 2, 128).transpose(
        1, 0, 2, 3).reshape(128, 512).astype(ml_dtypes.bfloat16)
    wb_pk[:, 1024:1030] = W3.reshape(2, 128, OUT).transpose(
        1, 0, 2).reshape(128, 2 * OUT).astype(ml_dtypes.bfloat16)
    wt_pk = np.zeros((128, WTAIL), np.float32)
    wt_pk[:, 0:2] = np.ascontiguousarray((OMEGA * b1).reshape(2, 128).T)
    wt_pk[:, 2:4] = np.ascontiguousarray((OMEGA * b2).reshape(2, 128).T)
    wt_pk[0:OUT, 4] = b3

    ys = np.linspace(-1.0, 1.0, H, dtype=np.float32)
    xcs = np.linspace(-1.0, 1.0, W, dtype=np.float32)

    shared = {"w0h": _e4(w0h_pk), "w0l": w0l_pk, "wt": wt_pk,
              "wb": wb_pk}
    in_maps = []
    for c in range(NCORES):
        m = _prep_core_inputs(c, xi, xcs, ys)
        m.update(shared)
        in_maps.append(m)

    nc = _build()
    res = run_bass_kernel_spmd(nc, in_maps, core_ids=list(range(NCORES)))
    global LAST_RES
    LAST_RES = res

    full = np.empty((B, OUT, H, W), np.float32)
    for c in range(NCORES):
        b = c // 4
        y0 = (c % 4) * ROWS
        full[b, :, y0:y0 + ROWS, :] = res.results[c]["out"].reshape(
            OUT, ROWS, W)
    return full
